# revision 1
# baseline (speedup 1.0000x reference)
"""CompressAttn Trainium2 Bass kernel.

Problem: compressed-block attention.
  B=2, N=4096, QH=32, KH=2, D=VD=128, KSZ=32, STRIDE=16, M=255 blocks.
  kc[b,m,h,:] = sum_i w_k[i] * (k[b,16m+i,h,:] + pe_k[i,:])   (same for v)
  out = softmax(q @ kc^T * D^-0.5, causal-banded mask) @ vc, zero for n < 31.

Sharding: 8 cores = (batch b in {0,1}) x (query-head quarter hq in {0..3}).
Each core handles 8 query heads that share a single KV head (g = hq//2), so
K/V compression is done once per core.  No collectives needed; host gathers.

Per-core device pipeline (all matmuls in float32r = full-rate fp32 path):
  1. Compression via banded matmul: for each 128-row chunk c of k (stationary)
     stream a constant [128,16] block-diag weight tile -> psum [d, (a,t)] with
     P_a[t] = sum_{s<16} w[16a+s] k[16t+s].  kcT[d,m] = P0[m]+P1[m+1]+bias_k.
     v likewise -> vcT, then PE-transpose to natural vc and append a ones
     column (PV then yields the softmax denominator for free).
  2. Per (head, 512-col block b): scoresT[m-chunk, 512] = kcT_chunk^T @ qT.
  3. exp on ScalarE (scale = D^-0.5 fused), multiplicative static staircase
     mask on the 32 diagonal band rows (b-independent [32,512] constant).
  4. Per 128-query tile: PV = expT_tile^T @ [vc|1|vc|1] (258 cols to stay on
     the fast fp32r path), accumulate 2 m-chunks in psum.  reciprocal of the
     ones column, ScalarE per-partition scale -> natural [n, vd] output tile.
"""

import ml_dtypes
import numpy as np

import concourse.bacc as bacc
import concourse.mybir as mybir
import concourse.tile as tile
from concourse.bass_utils import run_bass_kernel_spmd

# Problem geometry (hardcoded per contest rules).
B, N, QH, KH, D, VD = 2, 4096, 32, 2, 128, 128
KSZ, STRIDE = 32, 16
M = (N - KSZ) // STRIDE + 1          # 255 compressed blocks
T16 = N // STRIDE                     # 256 stride-16 sub-blocks
HPC = QH // 4                         # 8 query heads per core
NBLK = N // 512                       # 8 query blocks of 512
SM = float(D) ** -0.5

F32 = mybir.dt.float32
F32R = mybir.dt.float32r
BF16 = mybir.dt.bfloat16

# dtype switches for the two big matmul stages (float32r = single-pass fp32)
QK_DT = BF16
PV_DT = BF16
CP_DT = F32R


def _r(ap, dt):
    return ap


def build_program():
    nc = bacc.Bacc("TRN2", target_bir_lowering=False, debug=False)

    qT_d = nc.dram_tensor("qT", [HPC, D, N], QK_DT, kind="ExternalInput")
    k_d = nc.dram_tensor("kk", [N, D], CP_DT, kind="ExternalInput")
    v_d = nc.dram_tensor("vv", [N, D], CP_DT, kind="ExternalInput")
    w01k_d = nc.dram_tensor("w01k", [128, 16], CP_DT, kind="ExternalInput")
    w01v_d = nc.dram_tensor("w01v", [128, 16], CP_DT, kind="ExternalInput")
    bk_d = nc.dram_tensor("biask", [128, 1], F32, kind="ExternalInput")
    bv_d = nc.dram_tensor("biasv", [128, 1], F32, kind="ExternalInput")
    m01_d = nc.dram_tensor("m01", [8, 128, 512], PV_DT, kind="ExternalInput")
    id_d = nc.dram_tensor("ident", [128, 128], F32, kind="ExternalInput")
    ones_d = nc.dram_tensor("ones1", [128, 2], PV_DT, kind="ExternalInput")
    o_d = nc.dram_tensor("o", [HPC, N, VD], F32, kind="ExternalOutput")

    with tile.TileContext(nc) as tc:
        with tc.tile_pool(name="consts", bufs=1) as cp:
            w01k = cp.tile([128, 16], CP_DT)
            w01v = cp.tile([128, 16], CP_DT)
            biask = cp.tile([128, 1], F32)
            biasv = cp.tile([128, 1], F32)
            m01 = cp.tile([128, 8 * 512], PV_DT)
            ident = cp.tile([128, 128], F32)
            ktile = cp.tile([128, 32 * 128], CP_DT)
            vtile = cp.tile([128, 32 * 128], CP_DT)
            kcT = cp.tile([128, M], QK_DT)        # [d, m]
            vcT = cp.tile([128, 256], F32)      # [d, t] staging
            vca0 = cp.tile([128, 130], PV_DT)     # [m 0:128,   vc|1|0]
            vca1 = cp.tile([128, 130], PV_DT)     # [m 128:255, vc|1|0]

            nc.sync.dma_start(w01k[:, :], w01k_d.ap())
            nc.sync.dma_start(w01v[:, :], w01v_d.ap())
            nc.sync.dma_start(biask[:, :], bk_d.ap())
            nc.sync.dma_start(biasv[:, :], bv_d.ap())
            nc.sync.dma_start(
                m01[:, :].rearrange("p (j n) -> p j n", j=8),
                m01_d.ap().rearrange("j p n -> p j n"),
            )
            nc.sync.dma_start(ident[:, :], id_d.ap())
            nc.sync.dma_start(
                ktile[:, :].rearrange("p (c d) -> p c d", c=32),
                k_d.ap().rearrange("(c r) d -> r c d", r=128),
            )
            nc.sync.dma_start(
                vtile[:, :].rearrange("p (c d) -> p c d", c=32),
                v_d.ap().rearrange("(c r) d -> r c d", r=128),
            )

            # ---- compression ----
            with tc.tile_pool(name="ppsum", bufs=1, space="PSUM") as pp:
                # free layout (t, a): pkT[d, 2t+a] = P_a[t]
                pkT = pp.tile([128, 512], F32)
                pvT = pp.tile([128, 512], F32)
                tpA = pp.tile([128, 128], F32)
                tpB = pp.tile([128, 128], F32)
                for c in range(32):
                    nc.tensor.matmul(
                        pkT[:, 16 * c : 16 * c + 16],
                        _r(ktile[:, 128 * c : 128 * (c + 1)], CP_DT),
                        _r(w01k[:, :], CP_DT),
                        start=True, stop=True,
                    )
                    nc.tensor.matmul(
                        pvT[:, 16 * c : 16 * c + 16],
                        _r(vtile[:, 128 * c : 128 * (c + 1)], CP_DT),
                        _r(w01v[:, :], CP_DT),
                        start=True, stop=True,
                    )
                # kcT[d,m] = P0[m] + P1[m+1] + bias_k[d]
                pk3 = pkT[:, :].rearrange("p (t a) -> p t a", a=2)
                pv3 = pvT[:, :].rearrange("p (t a) -> p t a", a=2)
                # (walrus: only one PSUM input per DVE op -> two steps)
                nc.vector.tensor_scalar_add(kcT[:, 0:M], pk3[:, 0:M, 0], biask[:, 0:1])
                nc.vector.tensor_add(kcT[:, 0:M], kcT[:, 0:M], pk3[:, 1 : M + 1, 1])
                nc.vector.tensor_scalar_add(vcT[:, 0:M], pv3[:, 0:M, 0], biasv[:, 0:1])
                nc.vector.tensor_add(vcT[:, 0:M], vcT[:, 0:M], pv3[:, 1 : M + 1, 1])
                nc.vector.memset(vcT[:, M : M + 1], 0.0)
                # transpose vcT -> natural vc, build [vc|1|vc|1]
                nc.tensor.transpose(tpA[:, :], vcT[:, 0:128], ident[:, :])
                nc.tensor.transpose(tpB[:, :], vcT[:, 128:256], ident[:, :])
                nc.vector.tensor_copy(vca0[:, 0:128], tpA[:, :])
                nc.vector.tensor_copy(vca1[:, 0:128], tpB[:, :])
                nc.sync.dma_start(vca0[:, 128:130], ones_d.ap())
                nc.sync.dma_start(vca1[:, 128:130], ones_d.ap())

            # ---- attention ----
            with (
                tc.tile_pool(name="qp", bufs=2) as qp,
                tc.tile_pool(name="ep", bufs=4) as ep,
                tc.tile_pool(name="op", bufs=2) as op,
                tc.tile_pool(name="rp", bufs=8) as rp,
                tc.tile_pool(name="sps", bufs=4, space="PSUM") as sps,
                tc.tile_pool(name="pvs", bufs=2, space="PSUM") as pvs,
            ):
                for h in range(HPC):
                    qTh = qp.tile([128, N], QK_DT, tag="qTh")
                    nc.sync.dma_start(qTh[:, :], qT_d.ap()[h])
                    for b in range(NBLK):
                        mr = min(32 * b + 31, M)      # visible m count
                        c0r = min(mr, 128)
                        c1r = mr - 128
                        qs = qTh[:, 512 * b : 512 * (b + 1)]

                        sT0 = sps.tile([128, 512], F32, tag="sT")
                        nc.tensor.matmul(
                            sT0[0:c0r, :],
                            _r(kcT[:, 0:c0r], QK_DT),
                            _r(qs, QK_DT),
                            start=True, stop=True,
                        )
                        eT0 = ep.tile([128, 512], PV_DT, tag="eT")
                        nc.scalar.activation(
                            eT0[0:c0r, :], sT0[0:c0r, :],
                            mybir.ActivationFunctionType.Exp, scale=SM,
                        )
                        if c1r > 0:
                            sT1 = sps.tile([128, 512], F32, tag="sT")
                            nc.tensor.matmul(
                                sT1[0:c1r, :],
                                _r(kcT[:, 128 : 128 + c1r], QK_DT),
                                _r(qs, QK_DT),
                                start=True, stop=True,
                            )
                            eT1 = ep.tile([128, 512], PV_DT, tag="eT")
                            nc.scalar.activation(
                                eT1[0:c1r, :], sT1[0:c1r, :],
                                mybir.ActivationFunctionType.Exp, scale=SM,
                            )
                        # multiplicative staircase mask over the aligned
                        # 64-row window [32b-32, 32b+32); m01 row r covers
                        # m = 32b-32+r (visible iff n' >= 16r-481).
                        # staircase mask variant v holds stair[p-32v+32] at
                        # partition p, so both operands share base partitions
                        # (32-row pieces: non-zero-base APs cap at 32 rows).
                        w0 = 32 * b - 32
                        for ww in (w0, w0 + 32):
                            s0, e0 = max(ww, 0), min(ww + 32, c0r)
                            if s0 < e0:
                                mj = m01[:, 512 * b : 512 * (b + 1)]
                                nc.vector.tensor_mul(
                                    eT0[s0:e0, :], eT0[s0:e0, :], mj[s0:e0, :]
                                )
                            if c1r > 0:
                                s1 = max(ww, 128) - 128
                                e1 = min(ww + 32, 128 + c1r) - 128
                                if s1 < e1:
                                    mj = m01[:, 512 * (b - 4) : 512 * (b - 3)]
                                    nc.vector.tensor_mul(
                                        eT1[s1:e1, :], eT1[s1:e1, :], mj[s1:e1, :]
                                    )
                        o_blk = op.tile([128, 512], F32, tag="o")
                        for pr in range(2):
                            pvt = pvs.tile([128, 512], F32, tag="pv")
                            pv3 = pvt[:, 0:260].rearrange(
                                "p (j c) -> p j c", j=2
                            )
                            for j in range(2):
                                tt = 2 * pr + j
                                t = 4 * b + tt
                                K = 8 * t + 7
                                c0k = min(K, 128)
                                c1k = K - 128
                                out_ap = pvt[:, 130 * j : 130 * j + 130]
                                nc.tensor.matmul(
                                    out_ap,
                                    _r(eT0[0:c0k, 128 * tt : 128 * (tt + 1)], PV_DT),
                                    _r(vca0[0:c0k, :], PV_DT),
                                    start=True, stop=(c1k <= 0),
                                )
                                if c1k > 0:
                                    nc.tensor.matmul(
                                        out_ap,
                                        _r(eT1[0:c1k, 128 * tt : 128 * (tt + 1)], PV_DT),
                                        _r(vca1[0:c1k, :], PV_DT),
                                        start=False, stop=True,
                                    )
                            rc = rp.tile([128, 2], F32, tag="rc")
                            if b == 0 and pr == 0:
                                rtmp = rp.tile([128, 2], F32, tag="rtmp")
                                nc.vector.tensor_scalar_add(
                                    rtmp[:, :], pv3[:, :, 128], 1e-30
                                )
                                nc.vector.reciprocal(rc[:, :], rtmp[:, :])
                            else:
                                nc.vector.reciprocal(rc[:, :], pv3[:, :, 128])
                            for j in range(2):
                                tt = 2 * pr + j
                                dst = o_blk[:, 128 * tt : 128 * (tt + 1)]
                                src = pvt[:, 130 * j : 130 * j + 128]
                                if tt % 2 == 0:
                                    nc.scalar.mul(dst, src, rc[:, j : j + 1])
                                else:
                                    nc.vector.tensor_scalar_mul(
                                        dst, src, rc[:, j : j + 1]
                                    )
                        nc.sync.dma_start(
                            o_d.ap()[h, 512 * b : 512 * (b + 1), :].rearrange(
                                "(tt p) vd -> p tt vd", p=128
                            ),
                            o_blk[:, :].rearrange("p (tt vd) -> p tt vd", tt=4),
                        )
    nc.compile()
    return nc


def make_consts(w_k, pe_k, w_v, pe_v):
    """Host-side constant tensors fed to every core."""
    f = np.float32
    w01k = np.zeros((128, 16), f)
    w01v = np.zeros((128, 16), f)
    for r in range(128):
        j = r // 16
        s = r % 16
        for a in range(2):
            # column layout (j, a): col = 2*j + a, matching psum (t, a)
            w01k[r, 2 * j + a] = w_k[16 * a + s]
            w01v[r, 2 * j + a] = w_v[16 * a + s]
    biask = (w_k[:, None] * pe_k).sum(0).astype(f)[:, None]  # [128,1]
    biasv = (w_v[:, None] * pe_v).sum(0).astype(f)[:, None]
    # variant v: row p = stair(p - 32v + 32); stair(r): n' >= 16r - 481
    m01 = np.ones((8, 128, 512), f)
    for vv in range(8):
        for p in range(128):
            r = p - 32 * vv + 32
            if 0 <= r < 64:
                lo = 16 * r - 481
                if lo >= 512:
                    m01[vv, p, :] = 0.0
                else:
                    m01[vv, p, : max(lo, 0)] = 0.0
    ident = np.eye(128, dtype=f)
    return {
        "w01k": np.ascontiguousarray(w01k),
        "w01v": np.ascontiguousarray(w01v),
        "biask": np.ascontiguousarray(biask),
        "biasv": np.ascontiguousarray(biasv),
        "m01": m01.astype(ml_dtypes.bfloat16),
        "ident": ident,
        "ones1": np.hstack([np.ones((128, 1)), np.zeros((128, 1))]).astype(ml_dtypes.bfloat16),
    }


def make_in_map(q, k, v, consts, core):
    b, hq = core // 4, core % 4
    g = hq // 2
    qT = np.ascontiguousarray(
        q[b, :, 8 * hq : 8 * (hq + 1), :].transpose(1, 2, 0)
    ).astype(ml_dtypes.bfloat16)  # [8, D, N]
    return {
        "qT": qT,
        "kk": np.ascontiguousarray(k[b, :, g, :]),
        "vv": np.ascontiguousarray(v[b, :, g, :]),
        **consts,
    }


_CACHE = {}


def _compiled():
    if "nc" not in _CACHE:
        _CACHE["nc"] = build_program()
    return _CACHE["nc"]


def kernel(q, k, v, w_k, pe_k, w_v, pe_v, _trace=False, _trace_kwargs=None):
    q = np.asarray(q, np.float32)
    k = np.asarray(k, np.float32)
    v = np.asarray(v, np.float32)
    consts = make_consts(
        np.asarray(w_k, np.float32), np.asarray(pe_k, np.float32),
        np.asarray(w_v, np.float32), np.asarray(pe_v, np.float32),
    )
    nc = _compiled()
    in_maps = [make_in_map(q, k, v, consts, c) for c in range(8)]
    kw = {}
    if _trace:
        kw = {"trace": True, **(_trace_kwargs or {})}
    res = run_bass_kernel_spmd(nc, in_maps, core_ids=list(range(8)), **kw)
    out = np.empty((B, N, QH, VD), np.float32)
    for c in range(8):
        b, hq = c // 4, c % 4
        out[b, :, 8 * hq : 8 * (hq + 1), :] = res.results[c]["o"].transpose(1, 0, 2)
    _CACHE["last_result"] = res
    return out



# revision 6
# speedup vs baseline: 1.1919x; 1.1919x over previous
"""CompressAttn Trainium2 Bass kernel (v2: transposed PV + host normalize).

Problem: compressed-block attention.
  B=2, N=4096, QH=32, KH=2, D=VD=128, KSZ=32, STRIDE=16, M=255 blocks.
  kc[b,m,h,:] = sum_i w_k[i] * (k[b,16m+i,h,:] + pe_k[i,:])   (same for v)
  out = softmax(q @ kc^T * D^-0.5, causal-banded mask) @ vc, zero for n < 31.

Sharding: 8 cores = (batch b in {0,1}) x (query-head quarter hq in {0..3}).
Each core handles 8 query heads that share a single KV head (g = hq//2), so
K/V compression is done once per core.  No collectives needed; host gathers.

Device pipeline per core (all attention matmuls bf16, psum f32):
  1. Compression via banded matmul (bf16): per 128-row chunk c of k
     (stationary) stream [128,16] block-diag weight tile -> psum [d,(t,a)];
     kcT[d,m] = P0[m] + P1[m+1] + bias_k -> bf16.  v likewise -> vcT, then
     PE-transpose to natural vc0/vc1 [m, d] (PV stationaries).
  2. Per (head, 512-col block b): sT[m-chunk, 512] = kcT_chunk^T @ qT (1-2
     matmuls), exp on ScalarE (scale fused), multiplicative staircase mask
     on DVE (bf16).
  3. Transposed PV: oT[vd, 512] = vc_chunk^T(stationary) @ eT(moving),
     1-2 matmuls accumulated in psum.  Denominator row dn[1, 512] via a
     ones[mc,1] stationary matmul over the same eT moving.
  4. oT psum -> sbuf bf16 on GpSimd (1KB/partition DMA packets), dn -> f32.
     Softmax division happens on the host (o / max(dn, eps)); queries n<31
     have dn == 0 and o == 0 exactly.
"""

import ml_dtypes
import numpy as np

import concourse.bacc as bacc
import concourse.mybir as mybir
import concourse.tile as tile
from concourse.bass_utils import run_bass_kernel_spmd

# Problem geometry (hardcoded per contest rules).
B, N, QH, KH, D, VD = 2, 4096, 32, 2, 128, 128
KSZ, STRIDE = 32, 16
M = (N - KSZ) // STRIDE + 1          # 255 compressed blocks
HPC = QH // 4                         # 8 query heads per core
NBLK = N // 512                       # 8 query blocks of 512
SM = float(D) ** -0.5

F32 = mybir.dt.float32
BF16 = mybir.dt.bfloat16


def build_program():
    nc = bacc.Bacc("TRN2", target_bir_lowering=False, debug=False)

    qT_d = nc.dram_tensor("qT", [HPC, D, N], BF16, kind="ExternalInput")
    k_d = nc.dram_tensor("kk", [N, D], BF16, kind="ExternalInput")
    v_d = nc.dram_tensor("vv", [N, D], BF16, kind="ExternalInput")
    w01k_d = nc.dram_tensor("w01k", [128, 16], BF16, kind="ExternalInput")
    w01v_d = nc.dram_tensor("w01v", [128, 16], BF16, kind="ExternalInput")
    bk_d = nc.dram_tensor("biask", [128, 1], F32, kind="ExternalInput")
    bv_d = nc.dram_tensor("biasv", [128, 1], F32, kind="ExternalInput")
    m01_d = nc.dram_tensor("m01", [8, 128, 512], BF16, kind="ExternalInput")
    id_d = nc.dram_tensor("ident", [128, 128], BF16, kind="ExternalInput")
    o_d = nc.dram_tensor("o", [HPC, NBLK, VD, 512], BF16, kind="ExternalOutput")
    dn_d = nc.dram_tensor("dn", [HPC, NBLK, 1, 512], F32, kind="ExternalOutput")

    with tile.TileContext(nc) as tc:
        with tc.tile_pool(name="consts", bufs=1) as cp:
            w01k = cp.tile([128, 16], BF16)
            w01v = cp.tile([128, 16], BF16)
            biask = cp.tile([128, 1], F32)
            biasv = cp.tile([128, 1], F32)
            m01 = cp.tile([128, 8 * 512], BF16)
            ident = cp.tile([128, 128], BF16)
            onesc = cp.tile([128, 1], BF16)
            kcT = cp.tile([128, M], BF16)         # [d, m] QK stationary
            vcT = cp.tile([128, 256], BF16)       # [d, t] staging
            vc0 = cp.tile([128, 128], BF16)       # [m 0:128,   d]
            vc1 = cp.tile([128, 128], BF16)       # [m 128:255, d]

            nc.sync.dma_start(w01k[:, :], w01k_d.ap())
            nc.sync.dma_start(w01v[:, :], w01v_d.ap())
            nc.sync.dma_start(biask[:, :], bk_d.ap())
            nc.sync.dma_start(biasv[:, :], bv_d.ap())
            nc.sync.dma_start(
                m01[:, :].rearrange("p (j n) -> p j n", j=8),
                m01_d.ap().rearrange("j p n -> p j n"),
            )
            nc.sync.dma_start(ident[:, :], id_d.ap())
            nc.vector.memset(onesc[:, :], 1.0)

            # ---- compression ----
            with (
                tc.tile_pool(name="kvload", bufs=1) as kvp,
                tc.tile_pool(name="ppsum", bufs=1, space="PSUM") as pp,
            ):
                ktile = kvp.tile([128, 32 * 128], BF16)
                vtile = kvp.tile([128, 32 * 128], BF16)
                nc.sync.dma_start(
                    ktile[:, :].rearrange("p (c d) -> p c d", c=32),
                    k_d.ap().rearrange("(c r) d -> r c d", r=128),
                )
                nc.sync.dma_start(
                    vtile[:, :].rearrange("p (c d) -> p c d", c=32),
                    v_d.ap().rearrange("(c r) d -> r c d", r=128),
                )
                # free layout (t, a): pkT[d, 2t+a] = P_a[t]
                pkT = pp.tile([128, 512], F32)
                pvT = pp.tile([128, 512], F32)
                tpA = pp.tile([128, 128], BF16)
                tpB = pp.tile([128, 128], BF16)
                for c in range(32):
                    nc.tensor.matmul(
                        pkT[:, 16 * c : 16 * c + 16],
                        ktile[:, 128 * c : 128 * (c + 1)],
                        w01k[:, :],
                        start=True, stop=True,
                    )
                    nc.tensor.matmul(
                        pvT[:, 16 * c : 16 * c + 16],
                        vtile[:, 128 * c : 128 * (c + 1)],
                        w01v[:, :],
                        start=True, stop=True,
                    )
                # kcT[d,m] = P0[m] + P1[m+1] + bias_k[d]
                pk3 = pkT[:, :].rearrange("p (t a) -> p t a", a=2)
                pv3 = pvT[:, :].rearrange("p (t a) -> p t a", a=2)
                # (walrus: only one PSUM input per DVE op -> two steps)
                nc.vector.tensor_scalar_add(kcT[:, 0:M], pk3[:, 0:M, 0], biask[:, 0:1])
                nc.vector.tensor_add(kcT[:, 0:M], kcT[:, 0:M], pk3[:, 1 : M + 1, 1])
                nc.vector.tensor_scalar_add(vcT[:, 0:M], pv3[:, 0:M, 0], biasv[:, 0:1])
                nc.vector.tensor_add(vcT[:, 0:M], vcT[:, 0:M], pv3[:, 1 : M + 1, 1])
                nc.vector.memset(vcT[:, M : M + 1], 0.0)
                # transpose vcT -> natural vc [m, d] (PV-T stationaries)
                nc.tensor.transpose(tpA[:, :], vcT[:, 0:128], ident[:, :])
                nc.tensor.transpose(tpB[:, :], vcT[:, 128:256], ident[:, :])
                nc.vector.tensor_copy(vc0[:, :], tpA[:, :])
                nc.vector.tensor_copy(vc1[:, :], tpB[:, :])

            # ---- attention ----
            with (
                tc.tile_pool(name="qp", bufs=2) as qp,
                tc.tile_pool(name="ep", bufs=4) as ep,
                tc.tile_pool(name="op", bufs=3) as op,
                tc.tile_pool(name="dnp", bufs=3) as dnp,
                tc.tile_pool(name="sps", bufs=4, space="PSUM") as sps,
                tc.tile_pool(name="pvs", bufs=2, space="PSUM") as pvs,
                tc.tile_pool(name="dns", bufs=2, space="PSUM") as dns,
            ):
                for h in range(HPC):
                    qTh = qp.tile([128, N], BF16, tag="qTh")
                    nc.sync.dma_start(qTh[:, :], qT_d.ap()[h])
                    for b in range(NBLK):
                        mr = min(32 * b + 31, M)      # visible m count
                        c0r = min(mr, 128)
                        c1r = mr - 128
                        qs = qTh[:, 512 * b : 512 * (b + 1)]

                        # --- scores + exp + mask (per m-chunk) ---
                        sT0 = sps.tile([128, 512], F32, tag="sT")
                        nc.tensor.matmul(
                            sT0[0:c0r, :], kcT[:, 0:c0r], qs,
                            start=True, stop=True,
                        )
                        eT0 = ep.tile([128, 512], BF16, tag="eT")
                        nc.scalar.activation(
                            eT0[0:c0r, :], sT0[0:c0r, :],
                            mybir.ActivationFunctionType.Exp, scale=SM,
                        )
                        if c1r > 0:
                            sT1 = sps.tile([128, 512], F32, tag="sT")
                            nc.tensor.matmul(
                                sT1[0:c1r, :], kcT[:, 128 : 128 + c1r], qs,
                                start=True, stop=True,
                            )
                            eT1 = ep.tile([128, 512], BF16, tag="eT")
                            nc.scalar.activation(
                                eT1[0:c1r, :], sT1[0:c1r, :],
                                mybir.ActivationFunctionType.Exp, scale=SM,
                            )
                        # multiplicative staircase mask over the aligned
                        # 64-row window [32b-32, 32b+32); m01 variant v row p
                        # holds stair(p - 32v + 32) so operands share base
                        # partitions (32-row pieces: non-zero-base APs cap
                        # at 32 rows).
                        w0 = 32 * b - 32
                        for ww in (w0, w0 + 32):
                            s0, e0 = max(ww, 0), min(ww + 32, c0r)
                            if s0 < e0:
                                mj = m01[:, 512 * b : 512 * (b + 1)]
                                nc.vector.tensor_mul(
                                    eT0[s0:e0, :], eT0[s0:e0, :], mj[s0:e0, :]
                                )
                            if c1r > 0:
                                s1 = max(ww, 128) - 128
                                e1 = min(ww + 32, 128 + c1r) - 128
                                if s1 < e1:
                                    mj = m01[:, 512 * (b - 4) : 512 * (b - 3)]
                                    nc.vector.tensor_mul(
                                        eT1[s1:e1, :], eT1[s1:e1, :], mj[s1:e1, :]
                                    )

                        # --- denominator row: dn[1,512] = sum_m eT[m,:] ---
                        # 2 consecutive blocks share one psum bank, at
                        # partition bases 0/64 (matmul psum out base must be
                        # 0/32/64), so one DVE copy serves 2 blocks.
                        j = b % 2
                        if j == 0:
                            dnt = dns.tile([128, 512], F32, tag="dn")
                        drow = dnt[64 * j : 64 * j + 1, :]
                        nc.tensor.matmul(
                            drow, onesc[0:c0r, 0:1], eT0[0:c0r, :],
                            start=True, stop=(c1r <= 0),
                        )
                        if c1r > 0:
                            nc.tensor.matmul(
                                drow, onesc[0:c1r, 0:1], eT1[0:c1r, :],
                                start=False, stop=True,
                            )

                        # --- transposed PV: oT[vd,512] = vc^T @ eT ---
                        pvt = pvs.tile([128, 512], F32, tag="pv")
                        nc.tensor.matmul(
                            pvt[:, :], vc0[0:c0r, :], eT0[0:c0r, :],
                            start=True, stop=(c1r <= 0),
                        )
                        if c1r > 0:
                            nc.tensor.matmul(
                                pvt[:, :], vc1[0:c1r, :], eT1[0:c1r, :],
                                start=False, stop=True,
                            )

                        # --- psum -> sbuf, DMA out ---
                        o_blk = op.tile([128, 512], BF16, tag="o")
                        nc.vector.tensor_copy(o_blk[:, :], pvt[:, :])
                        nc.sync.dma_start(o_d.ap()[h, b], o_blk[:, :])
                        if j == 1:
                            dnsb = dnp.tile([65, 512], F32, tag="dnsb")
                            nc.vector.tensor_copy(dnsb[:, :], dnt[0:65, :])
                            for jj in range(2):
                                nc.sync.dma_start(
                                    dn_d.ap()[h, b - 1 + jj],
                                    dnsb[64 * jj : 64 * jj + 1, :],
                                )
    nc.compile()
    return nc


def make_consts(w_k, pe_k, w_v, pe_v):
    """Host-side constant tensors fed to every core."""
    f = np.float32
    bf = ml_dtypes.bfloat16
    w01k = np.zeros((128, 16), f)
    w01v = np.zeros((128, 16), f)
    for r in range(128):
        j = r // 16
        s = r % 16
        for a in range(2):
            # column layout (j, a): col = 2*j + a, matching psum (t, a)
            w01k[r, 2 * j + a] = w_k[16 * a + s]
            w01v[r, 2 * j + a] = w_v[16 * a + s]
    biask = (w_k[:, None] * pe_k).sum(0).astype(f)[:, None]  # [128,1]
    biasv = (w_v[:, None] * pe_v).sum(0).astype(f)[:, None]
    # variant v: row p = stair(p - 32v + 32); stair(r): n' >= 16r - 481
    m01 = np.ones((8, 128, 512), f)
    for vv in range(8):
        for p in range(128):
            r = p - 32 * vv + 32
            if 0 <= r < 64:
                lo = 16 * r - 481
                if lo >= 512:
                    m01[vv, p, :] = 0.0
                else:
                    m01[vv, p, : max(lo, 0)] = 0.0
    ident = np.eye(128, dtype=f)
    return {
        "w01k": w01k.astype(bf),
        "w01v": w01v.astype(bf),
        "biask": np.ascontiguousarray(biask),
        "biasv": np.ascontiguousarray(biasv),
        "m01": m01.astype(bf),
        "ident": ident.astype(bf),
    }


def make_in_map(q, k, v, consts, core):
    b, hq = core // 4, core % 4
    g = hq // 2
    bf = ml_dtypes.bfloat16
    qT = np.ascontiguousarray(
        q[b, :, 8 * hq : 8 * (hq + 1), :].transpose(1, 2, 0)
    ).astype(bf)  # [8, D, N]
    return {
        "qT": qT,
        "kk": np.ascontiguousarray(k[b, :, g, :]).astype(bf),
        "vv": np.ascontiguousarray(v[b, :, g, :]).astype(bf),
        **consts,
    }


_CACHE = {}


def _compiled():
    if "nc" not in _CACHE:
        _CACHE["nc"] = build_program()
    return _CACHE["nc"]


def kernel(q, k, v, w_k, pe_k, w_v, pe_v, _trace=False, _trace_kwargs=None):
    q = np.asarray(q, np.float32)
    k = np.asarray(k, np.float32)
    v = np.asarray(v, np.float32)
    consts = make_consts(
        np.asarray(w_k, np.float32), np.asarray(pe_k, np.float32),
        np.asarray(w_v, np.float32), np.asarray(pe_v, np.float32),
    )
    nc = _compiled()
    in_maps = [make_in_map(q, k, v, consts, c) for c in range(8)]
    kw = {}
    if _trace:
        kw = {"trace": True, **(_trace_kwargs or {})}
    res = run_bass_kernel_spmd(nc, in_maps, core_ids=list(range(8)), **kw)
    out = np.empty((B, N, QH, VD), np.float32)
    for c in range(8):
        b, hq = c // 4, c % 4
        oT = np.asarray(res.results[c]["o"], np.float32)      # [8,8,128,512]
        dn = np.asarray(res.results[c]["dn"], np.float32)     # [8,8,1,512]
        o = oT.transpose(0, 1, 3, 2).reshape(HPC, N, VD)      # [h, n, vd]
        d = dn.reshape(HPC, N)
        o /= np.maximum(d, 1e-30)[:, :, None]
        out[b, :, 8 * hq : 8 * (hq + 1), :] = o.transpose(1, 0, 2)
    _CACHE["last_result"] = res
    return out


# revision 9
# speedup vs baseline: 1.2292x; 1.0313x over previous
"""CompressAttn Trainium2 Bass kernel (v2: transposed PV + host normalize).

Problem: compressed-block attention.
  B=2, N=4096, QH=32, KH=2, D=VD=128, KSZ=32, STRIDE=16, M=255 blocks.
  kc[b,m,h,:] = sum_i w_k[i] * (k[b,16m+i,h,:] + pe_k[i,:])   (same for v)
  out = softmax(q @ kc^T * D^-0.5, causal-banded mask) @ vc, zero for n < 31.

Sharding: 8 cores = (batch b in {0,1}) x (query-head quarter hq in {0..3}).
Each core handles 8 query heads that share a single KV head (g = hq//2), so
K/V compression is done once per core.  No collectives needed; host gathers.

Device pipeline per core (all attention matmuls bf16, psum f32):
  1. Compression via banded matmul (bf16): per 128-row chunk c of k
     (stationary) stream [128,16] block-diag weight tile -> psum [d,(t,a)];
     kcT[d,m] = P0[m] + P1[m+1] + bias_k -> bf16.  v likewise -> vcT, then
     PE-transpose to natural vc0/vc1 [m, d] (PV stationaries).
  2. Per (head, 512-col block b): sT[m-chunk, 512] = kcT_chunk^T @ qT (1-2
     matmuls), exp on ScalarE (scale fused), multiplicative staircase mask
     on DVE (bf16).
  3. Transposed PV: oT[vd, 512] = vc_chunk^T(stationary) @ eT(moving),
     1-2 matmuls accumulated in psum.  Denominator row dn[1, 512] via a
     ones[mc,1] stationary matmul over the same eT moving.
  4. oT psum -> sbuf bf16 on GpSimd (1KB/partition DMA packets), dn -> f32.
     Softmax division happens on the host (o / max(dn, eps)); queries n<31
     have dn == 0 and o == 0 exactly.
"""

import ml_dtypes
import numpy as np

import concourse.bacc as bacc
import concourse.mybir as mybir
import concourse.tile as tile
from concourse.bass_utils import run_bass_kernel_spmd

# Problem geometry (hardcoded per contest rules).
B, N, QH, KH, D, VD = 2, 4096, 32, 2, 128, 128
KSZ, STRIDE = 32, 16
M = (N - KSZ) // STRIDE + 1          # 255 compressed blocks
HPC = QH // 4                         # 8 query heads per core
NBLK = N // 512                       # 8 query blocks of 512
SM = float(D) ** -0.5

F32 = mybir.dt.float32
BF16 = mybir.dt.bfloat16


def build_program():
    nc = bacc.Bacc("TRN2", target_bir_lowering=False, debug=False)

    qT_d = nc.dram_tensor("qT", [HPC, D, N], BF16, kind="ExternalInput")
    k_d = nc.dram_tensor("kk", [N, D], BF16, kind="ExternalInput")
    v_d = nc.dram_tensor("vv", [N, D], BF16, kind="ExternalInput")
    w01k_d = nc.dram_tensor("w01k", [128, 16], BF16, kind="ExternalInput")
    w01v_d = nc.dram_tensor("w01v", [128, 16], BF16, kind="ExternalInput")
    bk_d = nc.dram_tensor("biask", [128, 1], F32, kind="ExternalInput")
    bv_d = nc.dram_tensor("biasv", [128, 1], F32, kind="ExternalInput")
    m01_d = nc.dram_tensor("m01", [8, 128, 512], BF16, kind="ExternalInput")
    id_d = nc.dram_tensor("ident", [128, 128], BF16, kind="ExternalInput")
    o_d = nc.dram_tensor("o", [HPC, NBLK, VD, 512], BF16, kind="ExternalOutput")
    dn_d = nc.dram_tensor("dn", [HPC, NBLK, 1, 512], F32, kind="ExternalOutput")

    with tile.TileContext(nc) as tc:
        with tc.tile_pool(name="consts", bufs=1) as cp:
            w01k = cp.tile([128, 16], BF16)
            w01v = cp.tile([128, 16], BF16)
            biask = cp.tile([128, 1], F32)
            biasv = cp.tile([128, 1], F32)
            m01 = cp.tile([128, 8 * 512], BF16)
            ident = cp.tile([128, 128], BF16)
            onesc = cp.tile([128, 1], BF16)
            kcT = cp.tile([128, M], BF16)         # [d, m] QK stationary
            vcT = cp.tile([128, 256], BF16)       # [d, t] staging
            vc0 = cp.tile([128, 128], BF16)       # [m 0:128,   d]
            vc1 = cp.tile([128, 128], BF16)       # [m 128:255, d]

            nc.sync.dma_start(w01k[:, :], w01k_d.ap())
            nc.sync.dma_start(w01v[:, :], w01v_d.ap())
            nc.sync.dma_start(biask[:, :], bk_d.ap())
            nc.sync.dma_start(biasv[:, :], bv_d.ap())
            nc.sync.dma_start(
                m01[:, :].rearrange("p (j n) -> p j n", j=8),
                m01_d.ap().rearrange("j p n -> p j n"),
            )
            nc.sync.dma_start(ident[:, :], id_d.ap())
            nc.vector.memset(onesc[:, :], 1.0)

            # ---- compression ----
            with (
                tc.tile_pool(name="kvload", bufs=1) as kvp,
                tc.tile_pool(name="ppsum", bufs=1, space="PSUM") as pp,
            ):
                ktile = kvp.tile([128, 32 * 128], BF16)
                vtile = kvp.tile([128, 32 * 128], BF16)
                nc.sync.dma_start(
                    ktile[:, :].rearrange("p (c d) -> p c d", c=32),
                    k_d.ap().rearrange("(c r) d -> r c d", r=128),
                )
                nc.sync.dma_start(
                    vtile[:, :].rearrange("p (c d) -> p c d", c=32),
                    v_d.ap().rearrange("(c r) d -> r c d", r=128),
                )
                # free layout (t, a): pkT[d, 2t+a] = P_a[t]
                pkT = pp.tile([128, 512], F32)
                pvT = pp.tile([128, 512], F32)
                tpA = pp.tile([128, 128], BF16)
                tpB = pp.tile([128, 128], BF16)
                for c in range(32):
                    nc.tensor.matmul(
                        pkT[:, 16 * c : 16 * c + 16],
                        ktile[:, 128 * c : 128 * (c + 1)],
                        w01k[:, :],
                        start=True, stop=True,
                    )
                    nc.tensor.matmul(
                        pvT[:, 16 * c : 16 * c + 16],
                        vtile[:, 128 * c : 128 * (c + 1)],
                        w01v[:, :],
                        start=True, stop=True,
                    )
                # kcT[d,m] = P0[m] + P1[m+1] + bias_k[d]
                pk3 = pkT[:, :].rearrange("p (t a) -> p t a", a=2)
                pv3 = pvT[:, :].rearrange("p (t a) -> p t a", a=2)
                # (walrus: only one PSUM input per DVE op -> two steps)
                nc.vector.tensor_scalar_add(kcT[:, 0:M], pk3[:, 0:M, 0], biask[:, 0:1])
                nc.vector.tensor_add(kcT[:, 0:M], kcT[:, 0:M], pk3[:, 1 : M + 1, 1])
                nc.vector.tensor_scalar_add(vcT[:, 0:M], pv3[:, 0:M, 0], biasv[:, 0:1])
                nc.vector.tensor_add(vcT[:, 0:M], vcT[:, 0:M], pv3[:, 1 : M + 1, 1])
                nc.vector.memset(vcT[:, M : M + 1], 0.0)
                # transpose vcT -> natural vc [m, d] (PV-T stationaries)
                nc.tensor.transpose(tpA[:, :], vcT[:, 0:128], ident[:, :])
                nc.tensor.transpose(tpB[:, :], vcT[:, 128:256], ident[:, :])
                nc.vector.tensor_copy(vc0[:, :], tpA[:, :])
                nc.vector.tensor_copy(vc1[:, :], tpB[:, :])

            # ---- attention ----
            # Software-pipelined: stageA(i) = QK+exp+mask, stageB(i) =
            # dn+PV+copies+DMA.  stageB(i) is emitted after stageA(i+DEPTH)
            # so the in-order PE queue never waits on scalar/vector work of
            # the same iteration.
            with (
                tc.tile_pool(name="qp", bufs=2) as qp,
                tc.tile_pool(name="ep", bufs=6) as ep,
                tc.tile_pool(name="op", bufs=3) as op,
                tc.tile_pool(name="dnp", bufs=3) as dnp,
                tc.tile_pool(name="sps", bufs=4, space="PSUM") as sps,
                tc.tile_pool(name="pvs", bufs=2, space="PSUM") as pvs,
                tc.tile_pool(name="dns", bufs=2, space="PSUM") as dns,
            ):
                iters = [(h, b) for h in range(HPC) for b in range(NBLK)]
                st = {}
                qtiles = {}

                def load_q(h):
                    qtiles[h] = qp.tile([128, N], BF16, tag="qTh", name="qTh")
                    nc.sync.dma_start(qtiles[h][:, :], qT_d.ap()[h])

                def stageA(i):
                    h, b = iters[i]
                    if b == 0 and h + 1 < HPC:
                        load_q(h + 1)       # prefetch next head
                    mr = min(32 * b + 31, M)      # visible m count
                    c0r = min(mr, 128)
                    c1r = mr - 128
                    qs = qtiles[h][:, 512 * b : 512 * (b + 1)]

                    sT0 = sps.tile([128, 512], F32, tag="sT")
                    nc.tensor.matmul(
                        sT0[0:c0r, :], kcT[:, 0:c0r], qs,
                        start=True, stop=True,
                    )
                    eT0 = ep.tile([128, 512], BF16, tag="eT")
                    nc.scalar.activation(
                        eT0[0:c0r, :], sT0[0:c0r, :],
                        mybir.ActivationFunctionType.Exp, scale=SM,
                    )
                    eT1 = None
                    if c1r > 0:
                        sT1 = sps.tile([128, 512], F32, tag="sT")
                        nc.tensor.matmul(
                            sT1[0:c1r, :], kcT[:, 128 : 128 + c1r], qs,
                            start=True, stop=True,
                        )
                        eT1 = ep.tile([128, 512], BF16, tag="eT")
                        nc.scalar.activation(
                            eT1[0:c1r, :], sT1[0:c1r, :],
                            mybir.ActivationFunctionType.Exp, scale=SM,
                        )
                    # multiplicative staircase mask over the aligned 64-row
                    # window [32b-32, 32b+32); m01 variant v row p holds
                    # stair(p - 32v + 32) so operands share base partitions
                    # (32-row pieces: non-zero-base APs cap at 32 rows).
                    # Pieces alternate DVE / GpSimd to balance engine load.
                    w0 = 32 * b - 32
                    veng = [nc.vector, nc.gpsimd]
                    vi = 0
                    for ww in (w0, w0 + 32):
                        s0, e0 = max(ww, 0), min(ww + 32, c0r)
                        if s0 < e0:
                            mj = m01[:, 512 * b : 512 * (b + 1)]
                            veng[vi % 2].tensor_mul(
                                eT0[s0:e0, :], eT0[s0:e0, :], mj[s0:e0, :]
                            )
                            vi += 1
                        if c1r > 0:
                            s1 = max(ww, 128) - 128
                            e1 = min(ww + 32, 128 + c1r) - 128
                            if s1 < e1:
                                mj = m01[:, 512 * (b - 4) : 512 * (b - 3)]
                                veng[vi % 2].tensor_mul(
                                    eT1[s1:e1, :], eT1[s1:e1, :], mj[s1:e1, :]
                                )
                                vi += 1
                    st[i] = (c0r, c1r, eT0, eT1)

                def stageB(i):
                    h, b = iters[i]
                    c0r, c1r, eT0, eT1 = st.pop(i)

                    # denominator row dn[1,512] = sum_m eT[m,:]; 2
                    # consecutive blocks share one psum bank at partition
                    # bases 0/64 (matmul psum out base must be 0/32/64) so
                    # one DVE copy serves 2 blocks.
                    j = b % 2
                    if j == 0:
                        st[("dn", h)] = dns.tile(
                            [128, 512], F32, tag="dn", name="dnt"
                        )
                    dnt = st[("dn", h)]
                    drow = dnt[64 * j : 64 * j + 1, :]
                    nc.tensor.matmul(
                        drow, onesc[0:c0r, 0:1], eT0[0:c0r, :],
                        start=True, stop=(c1r <= 0),
                    )
                    if c1r > 0:
                        nc.tensor.matmul(
                            drow, onesc[0:c1r, 0:1], eT1[0:c1r, :],
                            start=False, stop=True,
                        )

                    # transposed PV: oT[vd,512] = vc^T @ eT
                    pvt = pvs.tile([128, 512], F32, tag="pv")
                    nc.tensor.matmul(
                        pvt[:, :], vc0[0:c0r, :], eT0[0:c0r, :],
                        start=True, stop=(c1r <= 0),
                    )
                    if c1r > 0:
                        nc.tensor.matmul(
                            pvt[:, :], vc1[0:c1r, :], eT1[0:c1r, :],
                            start=False, stop=True,
                        )

                    # psum -> sbuf, DMA out
                    o_blk = op.tile([128, 512], BF16, tag="o")
                    nc.vector.tensor_copy(o_blk[:, :], pvt[:, :])
                    nc.sync.dma_start(o_d.ap()[h, b], o_blk[:, :])
                    if j == 1:
                        dnsb = dnp.tile([65, 512], F32, tag="dnsb")
                        nc.vector.tensor_copy(dnsb[:, :], dnt[0:65, :])
                        for jj in range(2):
                            nc.sync.dma_start(
                                dn_d.ap()[h, b - 1 + jj],
                                dnsb[64 * jj : 64 * jj + 1, :],
                            )

                DEPTH = 2
                load_q(0)
                for i in range(len(iters)):
                    stageA(i)
                    if i >= DEPTH:
                        stageB(i - DEPTH)
                for i in range(len(iters) - DEPTH, len(iters)):
                    stageB(i)
    nc.compile()
    return nc


def make_consts(w_k, pe_k, w_v, pe_v):
    """Host-side constant tensors fed to every core."""
    f = np.float32
    bf = ml_dtypes.bfloat16
    w01k = np.zeros((128, 16), f)
    w01v = np.zeros((128, 16), f)
    for r in range(128):
        j = r // 16
        s = r % 16
        for a in range(2):
            # column layout (j, a): col = 2*j + a, matching psum (t, a)
            w01k[r, 2 * j + a] = w_k[16 * a + s]
            w01v[r, 2 * j + a] = w_v[16 * a + s]
    biask = (w_k[:, None] * pe_k).sum(0).astype(f)[:, None]  # [128,1]
    biasv = (w_v[:, None] * pe_v).sum(0).astype(f)[:, None]
    # variant v: row p = stair(p - 32v + 32); stair(r): n' >= 16r - 481
    m01 = np.ones((8, 128, 512), f)
    for vv in range(8):
        for p in range(128):
            r = p - 32 * vv + 32
            if 0 <= r < 64:
                lo = 16 * r - 481
                if lo >= 512:
                    m01[vv, p, :] = 0.0
                else:
                    m01[vv, p, : max(lo, 0)] = 0.0
    ident = np.eye(128, dtype=f)
    return {
        "w01k": w01k.astype(bf),
        "w01v": w01v.astype(bf),
        "biask": np.ascontiguousarray(biask),
        "biasv": np.ascontiguousarray(biasv),
        "m01": m01.astype(bf),
        "ident": ident.astype(bf),
    }


def make_in_map(q, k, v, consts, core):
    b, hq = core // 4, core % 4
    g = hq // 2
    bf = ml_dtypes.bfloat16
    qT = np.ascontiguousarray(
        q[b, :, 8 * hq : 8 * (hq + 1), :].transpose(1, 2, 0)
    ).astype(bf)  # [8, D, N]
    return {
        "qT": qT,
        "kk": np.ascontiguousarray(k[b, :, g, :]).astype(bf),
        "vv": np.ascontiguousarray(v[b, :, g, :]).astype(bf),
        **consts,
    }


_CACHE = {}


def _compiled():
    if "nc" not in _CACHE:
        _CACHE["nc"] = build_program()
    return _CACHE["nc"]


def kernel(q, k, v, w_k, pe_k, w_v, pe_v, _trace=False, _trace_kwargs=None):
    q = np.asarray(q, np.float32)
    k = np.asarray(k, np.float32)
    v = np.asarray(v, np.float32)
    consts = make_consts(
        np.asarray(w_k, np.float32), np.asarray(pe_k, np.float32),
        np.asarray(w_v, np.float32), np.asarray(pe_v, np.float32),
    )
    nc = _compiled()
    in_maps = [make_in_map(q, k, v, consts, c) for c in range(8)]
    kw = {}
    if _trace:
        kw = {"trace": True, **(_trace_kwargs or {})}
    res = run_bass_kernel_spmd(nc, in_maps, core_ids=list(range(8)), **kw)
    out = np.empty((B, N, QH, VD), np.float32)
    for c in range(8):
        b, hq = c // 4, c % 4
        oT = np.asarray(res.results[c]["o"], np.float32)      # [8,8,128,512]
        dn = np.asarray(res.results[c]["dn"], np.float32)     # [8,8,1,512]
        o = oT.transpose(0, 1, 3, 2).reshape(HPC, N, VD)      # [h, n, vd]
        d = dn.reshape(HPC, N)
        o /= np.maximum(d, 1e-30)[:, :, None]
        out[b, :, 8 * hq : 8 * (hq + 1), :] = o.transpose(1, 0, 2)
    _CACHE["last_result"] = res
    return out


# revision 23
# speedup vs baseline: 1.7203x; 1.3995x over previous
"""CompressAttn Trainium2 Bass kernel (v2: transposed PV + host normalize).

Problem: compressed-block attention.
  B=2, N=4096, QH=32, KH=2, D=VD=128, KSZ=32, STRIDE=16, M=255 blocks.
  kc[b,m,h,:] = sum_i w_k[i] * (k[b,16m+i,h,:] + pe_k[i,:])   (same for v)
  out = softmax(q @ kc^T * D^-0.5, causal-banded mask) @ vc, zero for n < 31.

Sharding: 8 cores = (batch b in {0,1}) x (query-head quarter hq in {0..3}).
Each core handles 8 query heads that share a single KV head (g = hq//2), so
K/V compression is done once per core.  No collectives needed; host gathers.

Device pipeline per core (all attention matmuls bf16, psum f32):
  1. Compression via banded matmul (bf16): per 128-row chunk c of k
     (stationary) stream [128,16] block-diag weight tile -> psum [d,(t,a)];
     kcT[d,m] = P0[m] + P1[m+1] + bias_k -> bf16.  v likewise -> vcT, then
     PE-transpose to natural vc0/vc1 [m, d] (PV stationaries).
  2. Per (head, 512-col block b): sT[m-chunk, 512] = kcT_chunk^T @ qT (1-2
     matmuls), exp on ScalarE (scale fused), multiplicative staircase mask
     on DVE (bf16).
  3. Transposed PV: oT[vd, 512] = vc_chunk^T(stationary) @ eT(moving),
     1-2 matmuls accumulated in psum.  Denominator row dn[1, 512] via a
     ones[mc,1] stationary matmul over the same eT moving.
  4. oT psum -> sbuf bf16 on GpSimd (1KB/partition DMA packets), dn -> f32.
     Softmax division happens on the host (o / max(dn, eps)); queries n<31
     have dn == 0 and o == 0 exactly.
"""

import ml_dtypes
import numpy as np

import concourse.bacc as bacc
import concourse.mybir as mybir
import concourse.tile as tile
from concourse.bass_utils import run_bass_kernel_spmd

# Problem geometry (hardcoded per contest rules).
B, N, QH, KH, D, VD = 2, 4096, 32, 2, 128, 128
KSZ, STRIDE = 32, 16
M = (N - KSZ) // STRIDE + 1          # 255 compressed blocks
HPC = QH // 4                         # 8 query heads per core
NBLK = N // 512                       # 8 query blocks of 512
SM = float(D) ** -0.5

F32 = mybir.dt.float32
BF16 = mybir.dt.bfloat16


def build_program():
    nc = bacc.Bacc("TRN2", target_bir_lowering=False, debug=False)

    qT_d = nc.dram_tensor("qT", [HPC, D, N], BF16, kind="ExternalInput")
    k_d = nc.dram_tensor("kk", [N, D], BF16, kind="ExternalInput")
    v_d = nc.dram_tensor("vv", [N, D], BF16, kind="ExternalInput")
    w01k_d = nc.dram_tensor("w01k", [128, 16], BF16, kind="ExternalInput")
    w01v_d = nc.dram_tensor("w01v", [128, 16], BF16, kind="ExternalInput")
    bk_d = nc.dram_tensor("biask", [128, 1], F32, kind="ExternalInput")
    bv_d = nc.dram_tensor("biasv", [128, 1], F32, kind="ExternalInput")
    m01_d = nc.dram_tensor("m01", [8, 128, 512], BF16, kind="ExternalInput")
    m01r_d = nc.dram_tensor("m01r", [128, 16], BF16, kind="ExternalInput")
    id_d = nc.dram_tensor("ident", [128, 128], BF16, kind="ExternalInput")
    o_d = nc.dram_tensor(
        "o", [HPC, NBLK // 2, VD, 1024], BF16, kind="ExternalOutput"
    )
    dn_d = nc.dram_tensor("dn", [HPC, NBLK, 1, 512], F32, kind="ExternalOutput")

    with tile.TileContext(nc) as tc:
        with tc.tile_pool(name="consts", bufs=1) as cp:
            w01k = cp.tile([128, 16], BF16)
            w01v = cp.tile([128, 16], BF16)
            biask = cp.tile([128, 1], F32)
            biasv = cp.tile([128, 1], F32)
            m01 = cp.tile([128, 8 * 512], BF16)
            m01r = cp.tile([128, 16], BF16)       # row p%32==31 -> 0, else 1
            ident = cp.tile([128, 128], BF16)
            onesc = cp.tile([128, 1], BF16)
            kcT = cp.tile([128, M], BF16)         # [d, m] QK stationary
            vcT = cp.tile([128, 256], BF16)       # [d, t] staging
            vc0 = cp.tile([128, 128], BF16)       # [m 0:128,   d]
            vc1 = cp.tile([128, 128], BF16)       # [m 128:255, d]

            # ---- attention (+ compression interleaved into the prologue) --
            # Software-pipelined: stageA(i) = QK+exp+mask, stageB(i) =
            # dn+PV+copies+DMA.  stageB(i) is emitted after stageA(i+DEPTH)
            # so the in-order PE queue never waits on scalar/vector work of
            # the same iteration.
            with (
                tc.tile_pool(name="kvload", bufs=1) as kvp,
                tc.tile_pool(name="qp", bufs=2) as qp,
                tc.tile_pool(name="ep", bufs=6) as ep,
                tc.tile_pool(name="op", bufs=3) as op,
                tc.tile_pool(name="dnp", bufs=3) as dnp,
                tc.tile_pool(name="sps", bufs=4, space="PSUM") as sps,
                tc.tile_pool(name="pvs", bufs=2, space="PSUM") as pvs,
                tc.tile_pool(name="dns", bufs=2, space="PSUM") as dns,
            ):
                iters = [(h, b) for h in range(HPC) for b in range(NBLK)]
                st = {}
                qtiles = {}

                def load_q(h):
                    qtiles[h] = qp.tile([128, N], BF16, tag="qTh", name="qTh")
                    nc.sync.dma_start(qtiles[h][:, :], qT_d.ap()[h])

                # qT(0) first so the first QK isn't blocked behind the
                # k/v/m01 loads on the sync queue.
                load_q(0)
                ktile = kvp.tile([128, 32 * 128], BF16)
                vtile = kvp.tile([128, 32 * 128], BF16)
                nc.sync.dma_start(
                    ktile[:, :].rearrange("p (c d) -> p c d", c=32),
                    k_d.ap().rearrange("(c r) d -> r c d", r=128),
                )
                nc.sync.dma_start(
                    vtile[:, :].rearrange("p (c d) -> p c d", c=32),
                    v_d.ap().rearrange("(c r) d -> r c d", r=128),
                )
                nc.sync.dma_start(w01k[:, :], w01k_d.ap())
                nc.sync.dma_start(w01v[:, :], w01v_d.ap())
                nc.sync.dma_start(biask[:, :], bk_d.ap())
                nc.sync.dma_start(biasv[:, :], bv_d.ap())
                nc.sync.dma_start(
                    m01[:, :].rearrange("p (j n) -> p j n", j=8),
                    m01_d.ap().rearrange("j p n -> p j n"),
                )
                nc.sync.dma_start(ident[:, :], id_d.ap())
                nc.sync.dma_start(m01r[:, :], m01r_d.ap())
                nc.vector.memset(onesc[:, :], 1.0)

                # ---- compression ----
                # free layout (t, a): pkT[d, 2t+a] = P_a[t].  Prologue psum
                # comes from the sps pool (same shape/tag) so everything
                # fits the 8 psum banks; the transposes reuse pkT/pvT's
                # banks via tag cycling, with a bf16 bitcast view.
                pkT = sps.tile([128, 512], F32, tag="sT", name="pkT")
                pvT = sps.tile([128, 512], F32, tag="sT", name="pvT")
                for c in range(32):
                    nc.tensor.matmul(
                        pkT[:, 16 * c : 16 * c + 16],
                        ktile[:, 128 * c : 128 * (c + 1)],
                        w01k[:, :],
                        start=True, stop=True,
                    )
                # kcT[d,m] = P0[m] + P1[m+1] + bias_k[d]; these DVE adds run
                # while the PE does the v matmuls below.
                pk3 = pkT[:, :].rearrange("p (t a) -> p t a", a=2)
                nc.vector.tensor_scalar_add(kcT[:, 0:M], pk3[:, 0:M, 0], biask[:, 0:1])
                nc.vector.tensor_add(kcT[:, 0:M], kcT[:, 0:M], pk3[:, 1 : M + 1, 1])
                for c in range(32):
                    nc.tensor.matmul(
                        pvT[:, 16 * c : 16 * c + 16],
                        vtile[:, 128 * c : 128 * (c + 1)],
                        w01v[:, :],
                        start=True, stop=True,
                    )
                pv3 = pvT[:, :].rearrange("p (t a) -> p t a", a=2)
                nc.vector.tensor_scalar_add(vcT[:, 0:M], pv3[:, 0:M, 0], biasv[:, 0:1])
                nc.vector.tensor_add(vcT[:, 0:M], vcT[:, 0:M], pv3[:, 1 : M + 1, 1])
                nc.vector.memset(vcT[:, M : M + 1], 0.0)

                def finish_compression():
                    # transpose vcT -> natural vc [m, d] (PV-T stationaries);
                    # emitted after the first two stageA's so QK(0) doesn't
                    # wait behind the transposes' vcT dependency.
                    tpA = sps.tile([128, 512], F32, tag="sT", name="tpA")
                    tpB = sps.tile([128, 512], F32, tag="sT", name="tpB")
                    tpAv = tpA.bitcast(BF16)[:, 0:128]
                    tpBv = tpB.bitcast(BF16)[:, 0:128]
                    nc.tensor.transpose(tpAv, vcT[:, 0:128], ident[:, :])
                    nc.tensor.transpose(tpBv, vcT[:, 128:256], ident[:, :])
                    nc.vector.tensor_copy(vc0[:, :], tpAv)
                    nc.vector.tensor_copy(vc1[:, :], tpBv)

                def stageA(i):
                    h, b = iters[i]
                    if b == 0 and h + 1 < HPC:
                        load_q(h + 1)       # prefetch next head
                    mr = min(32 * b + 31, M)      # visible m count
                    c0r = min(mr, 128)
                    c1r = mr - 128
                    qs = qtiles[h][:, 512 * b : 512 * (b + 1)]

                    sT0 = sps.tile([128, 512], F32, tag="sT")
                    nc.tensor.matmul(
                        sT0[0:c0r, :], kcT[:, 0:c0r], qs,
                        start=True, stop=True,
                    )
                    eT0 = ep.tile([128, 512], BF16, tag="eT")
                    nc.scalar.activation(
                        eT0[0:c0r, :], sT0[0:c0r, :],
                        mybir.ActivationFunctionType.Exp, scale=SM,
                    )
                    eT1 = None
                    if c1r > 0:
                        sT1 = sps.tile([128, 512], F32, tag="sT")
                        nc.tensor.matmul(
                            sT1[0:c1r, :], kcT[:, 128 : 128 + c1r], qs,
                            start=True, stop=True,
                        )
                        eT1 = ep.tile([128, 512], BF16, tag="eT")
                        nc.scalar.activation(
                            eT1[0:c1r, :], sT1[0:c1r, :],
                            mybir.ActivationFunctionType.Exp, scale=SM,
                        )
    # multiplicative staircase mask: only rows [32b, 32b+31) are
                    # partially masked (one aligned 31-row DVE mul with the
                    # matching m01 variant rows); row 32b-1 just zeroes its
                    # first 15 cols (GpSimd memset).  All other computed rows
                    # are fully visible.
                    ps, pe_ = 32 * b, 32 * b + 31
                    if pe_ <= 128:            # piece lives in chunk0
                        mj = m01[:, 512 * b : 512 * (b + 1)]
                        nc.vector.tensor_mul(
                            eT0[ps:pe_, :], eT0[ps:pe_, :], mj[ps:pe_, :]
                        )
                    else:                     # chunk1 (b >= 4)
                        s1, e1 = ps - 128, pe_ - 128
                        mj = m01[:, 512 * (b - 4) : 512 * (b - 3)]
                        nc.vector.tensor_mul(
                            eT1[s1:e1, :], eT1[s1:e1, :], mj[s1:e1, :]
                        )
                    # row 32b-1 masks cols 0:15; it is the last row of the
                    # aligned window [32b-32, 32b), so multiply that window's
                    # first 15 cols by m01r (zero only in rows p%32==31).
                    pr = 32 * b - 1
                    if 0 <= pr < 128:
                        ws = pr - 31
                        nc.gpsimd.tensor_mul(
                            eT0[ws : ws + 32, 0:15], eT0[ws : ws + 32, 0:15],
                            m01r[ws : ws + 32, 0:15],
                        )
                    elif pr >= 128:
                        ws = pr - 159
                        nc.gpsimd.tensor_mul(
                            eT1[ws : ws + 32, 0:15], eT1[ws : ws + 32, 0:15],
                            m01r[ws : ws + 32, 0:15],
                        )
                    st[i] = (c0r, c1r, eT0, eT1)

                def stageB(i):
                    h, b = iters[i]
                    c0r, c1r, eT0, eT1 = st.pop(i)

                    # denominator row dn[1,512] = sum_m eT[m,:]; 2
                    # consecutive blocks share one psum bank at partition
                    # bases 0/64 (matmul psum out base must be 0/32/64) so
                    # one DVE copy serves 2 blocks.
                    j = b % 2
                    if j == 0:
                        st[("dn", h)] = dns.tile(
                            [128, 512], F32, tag="dn", name="dnt"
                        )
                    dnt = st[("dn", h)]
                    drow = dnt[64 * j : 64 * j + 1, :]
                    nc.tensor.matmul(
                        drow, onesc[0:c0r, 0:1], eT0[0:c0r, :],
                        start=True, stop=(c1r <= 0),
                    )
                    if c1r > 0:
                        nc.tensor.matmul(
                            drow, onesc[0:c1r, 0:1], eT1[0:c1r, :],
                            start=False, stop=True,
                        )

                    # transposed PV: oT[vd,512] = vc^T @ eT
                    pvt = pvs.tile([128, 512], F32, tag="pv")
                    nc.tensor.matmul(
                        pvt[:, :], vc0[0:c0r, :], eT0[0:c0r, :],
                        start=True, stop=(c1r <= 0),
                    )
                    if c1r > 0:
                        nc.tensor.matmul(
                            pvt[:, :], vc1[0:c1r, :], eT1[0:c1r, :],
                            start=False, stop=True,
                        )

                    # psum -> sbuf; 2 consecutive blocks share one [128,1024]
                    # sbuf tile and a single 2KB-per-partition DMA.
                    if j == 0:
                        st[("o", h)] = op.tile(
                            [128, 1024], BF16, tag="o", name="o_blk"
                        )
                    o_blk = st[("o", h)]
                    nc.vector.tensor_copy(
                        o_blk[:, 512 * j : 512 * (j + 1)], pvt[:, :]
                    )
                    if j == 1:
                        nc.sync.dma_start(o_d.ap()[h, b // 2], o_blk[:, :])
                        dnsb = dnp.tile([65, 512], F32, tag="dnsb")
                        nc.vector.tensor_copy(dnsb[:, :], dnt[0:65, :])
                        for jj in range(2):
                            nc.sync.dma_start(
                                dn_d.ap()[h, b - 1 + jj],
                                dnsb[64 * jj : 64 * jj + 1, :],
                            )

                DEPTH = 2
                for i in range(len(iters)):
                    stageA(i)
                    if i == DEPTH - 1:
                        finish_compression()
                    if i >= DEPTH:
                        stageB(i - DEPTH)
                for i in range(len(iters) - DEPTH, len(iters)):
                    stageB(i)
    nc.compile()
    return nc


def make_consts(w_k, pe_k, w_v, pe_v):
    """Host-side constant tensors fed to every core."""
    f = np.float32
    bf = ml_dtypes.bfloat16
    w01k = np.zeros((128, 16), f)
    w01v = np.zeros((128, 16), f)
    for r in range(128):
        j = r // 16
        s = r % 16
        for a in range(2):
            # column layout (j, a): col = 2*j + a, matching psum (t, a)
            w01k[r, 2 * j + a] = w_k[16 * a + s]
            w01v[r, 2 * j + a] = w_v[16 * a + s]
    biask = (w_k[:, None] * pe_k).sum(0).astype(f)[:, None]  # [128,1]
    biasv = (w_v[:, None] * pe_v).sum(0).astype(f)[:, None]
    # variant v: row p = stair(p - 32v + 32); stair(r): n' >= 16r - 481
    m01 = np.ones((8, 128, 512), f)
    for vv in range(8):
        for p in range(128):
            r = p - 32 * vv + 32
            if 0 <= r < 64:
                lo = 16 * r - 481
                if lo >= 512:
                    m01[vv, p, :] = 0.0
                else:
                    m01[vv, p, : max(lo, 0)] = 0.0
    ident = np.eye(128, dtype=f)
    m01r = np.ones((128, 16), f)
    m01r[31::32, :] = 0.0
    return {
        "w01k": w01k.astype(bf),
        "w01v": w01v.astype(bf),
        "biask": np.ascontiguousarray(biask),
        "biasv": np.ascontiguousarray(biasv),
        "m01": m01.astype(bf),
        "m01r": m01r.astype(bf),
        "ident": ident.astype(bf),
    }


def make_in_map(q, k, v, consts, core):
    b, hq = core // 4, core % 4
    g = hq // 2
    bf = ml_dtypes.bfloat16
    qT = np.ascontiguousarray(
        q[b, :, 8 * hq : 8 * (hq + 1), :].transpose(1, 2, 0)
    ).astype(bf)  # [8, D, N]
    return {
        "qT": qT,
        "kk": np.ascontiguousarray(k[b, :, g, :]).astype(bf),
        "vv": np.ascontiguousarray(v[b, :, g, :]).astype(bf),
        **consts,
    }


_CACHE = {}


def _compiled():
    if "nc" not in _CACHE:
        _CACHE["nc"] = build_program()
    return _CACHE["nc"]


def kernel(q, k, v, w_k, pe_k, w_v, pe_v, _trace=False, _trace_kwargs=None):
    q = np.asarray(q, np.float32)
    k = np.asarray(k, np.float32)
    v = np.asarray(v, np.float32)
    consts = make_consts(
        np.asarray(w_k, np.float32), np.asarray(pe_k, np.float32),
        np.asarray(w_v, np.float32), np.asarray(pe_v, np.float32),
    )
    nc = _compiled()
    in_maps = [make_in_map(q, k, v, consts, c) for c in range(8)]
    kw = {}
    if _trace:
        kw = {"trace": True, **(_trace_kwargs or {})}
    res = run_bass_kernel_spmd(nc, in_maps, core_ids=list(range(8)), **kw)
    out = np.empty((B, N, QH, VD), np.float32)
    for c in range(8):
        b, hq = c // 4, c % 4
        oT = np.asarray(res.results[c]["o"], np.float32)      # [8,8,128,512]
        dn = np.asarray(res.results[c]["dn"], np.float32)     # [8,8,1,512]
        o = oT.transpose(0, 1, 3, 2).reshape(HPC, N, VD)      # [h, n, vd]
        d = dn.reshape(HPC, N)
        o /= np.maximum(d, 1e-30)[:, :, None]
        out[b, :, 8 * hq : 8 * (hq + 1), :] = o.transpose(1, 0, 2)
    _CACHE["last_result"] = res
    return out


# revision 31
# speedup vs baseline: 1.7944x; 1.0431x over previous
"""CompressAttn Trainium2 Bass kernel (v2: transposed PV + host normalize).

Problem: compressed-block attention.
  B=2, N=4096, QH=32, KH=2, D=VD=128, KSZ=32, STRIDE=16, M=255 blocks.
  kc[b,m,h,:] = sum_i w_k[i] * (k[b,16m+i,h,:] + pe_k[i,:])   (same for v)
  out = softmax(q @ kc^T * D^-0.5, causal-banded mask) @ vc, zero for n < 31.

Sharding: 8 cores = (batch b in {0,1}) x (query-head quarter hq in {0..3}).
Each core handles 8 query heads that share a single KV head (g = hq//2), so
K/V compression is done once per core.  No collectives needed; host gathers.

Device pipeline per core (all attention matmuls bf16, psum f32):
  1. Compression via banded matmul (bf16): per 128-row chunk c of k
     (stationary) stream [128,16] block-diag weight tile -> psum [d,(t,a)];
     kcT[d,m] = P0[m] + P1[m+1] + bias_k -> bf16.  v likewise -> vcT, then
     PE-transpose to natural vc0/vc1 [m, d] (PV stationaries).
  2. Per (head, 512-col block b): sT[m-chunk, 512] = kcT_chunk^T @ qT (1-2
     matmuls), exp on ScalarE (scale fused), multiplicative staircase mask
     on DVE (bf16).
  3. Transposed PV: oT[vd, 512] = vc_chunk^T(stationary) @ eT(moving),
     1-2 matmuls accumulated in psum.  Denominator row dn[1, 512] via a
     ones[mc,1] stationary matmul over the same eT moving.
  4. oT psum -> sbuf bf16 on GpSimd (1KB/partition DMA packets), dn -> f32.
     Softmax division happens on the host (o / max(dn, eps)); queries n<31
     have dn == 0 and o == 0 exactly.
"""

import ml_dtypes
import numpy as np

import concourse.bacc as bacc
import concourse.mybir as mybir
import concourse.tile as tile
from concourse.bass_utils import run_bass_kernel_spmd

# Problem geometry (hardcoded per contest rules).
B, N, QH, KH, D, VD = 2, 4096, 32, 2, 128, 128
KSZ, STRIDE = 32, 16
M = (N - KSZ) // STRIDE + 1          # 255 compressed blocks
HPC = QH // 4                         # 8 query heads per core
NBLK = N // 512                       # 8 query blocks of 512
SM = float(D) ** -0.5

F32 = mybir.dt.float32
BF16 = mybir.dt.bfloat16


def build_program():
    nc = bacc.Bacc("TRN2", target_bir_lowering=False, debug=False)

    # All inputs are host-pre-arranged so every DMA is contiguous per
    # partition (few, large descriptors — sync-queue descgen is expensive).
    qT_d = nc.dram_tensor("qT", [HPC, D, N], BF16, kind="ExternalInput")
    k_d = nc.dram_tensor("kk", [128, 32 * 128], BF16, kind="ExternalInput")
    v_d = nc.dram_tensor("vv", [128, 32 * 128], BF16, kind="ExternalInput")
    # bf16 blob cols: w01k[0:16] | w01v[16:32] | m01r[32:48] | ident[48:176]
    blob_d = nc.dram_tensor("blob", [128, 176], BF16, kind="ExternalInput")
    bias_d = nc.dram_tensor("biaskv", [128, 2], F32, kind="ExternalInput")
    m01_d = nc.dram_tensor("m01", [128, 8 * 512], BF16, kind="ExternalInput")
    o_d = nc.dram_tensor(
        "o", [HPC, NBLK // 4, VD, 2048], BF16, kind="ExternalOutput"
    )
    dn_d = nc.dram_tensor("dn", [HPC, 2, 4 * 512], F32, kind="ExternalOutput")

    with tile.TileContext(nc) as tc:
        with tc.tile_pool(name="consts", bufs=1) as cp:
            blob = cp.tile([128, 176], BF16)
            biaskv = cp.tile([128, 2], F32)
            m01 = cp.tile([128, 8 * 512], BF16)
            onesc = cp.tile([128, 1], BF16)
            kcT = cp.tile([128, M], BF16)         # [d, m] QK stationary
            vcT = cp.tile([128, 256], BF16)       # [d, t] staging
            vc0 = cp.tile([128, 128], BF16)       # [m 0:128,   d]
            vc1 = cp.tile([128, 128], BF16)       # [m 128:255, d]
            w01k = blob[:, 0:16]
            w01v = blob[:, 16:32]
            m01r = blob[:, 32:48]      # row p%32==31 -> 0, else 1
            ident = blob[:, 48:176]
            biask = biaskv[:, 0:1]
            biasv = biaskv[:, 1:2]

            # ---- attention (+ compression interleaved into the prologue) --
            # Software-pipelined: stageA(i) = QK+exp+mask, stageB(i) =
            # dn+PV+copies+DMA.  stageB(i) is emitted after stageA(i+DEPTH)
            # so the in-order PE queue never waits on scalar/vector work of
            # the same iteration.
            with (
                tc.tile_pool(name="kvload", bufs=1) as kvp,
                tc.tile_pool(name="qp", bufs=2) as qp,
                tc.tile_pool(name="ep", bufs=6) as ep,
                tc.tile_pool(name="op", bufs=3) as op,
                tc.tile_pool(name="dnp", bufs=3) as dnp,
                tc.tile_pool(name="sps", bufs=4, space="PSUM") as sps,
                tc.tile_pool(name="pvs", bufs=2, space="PSUM") as pvs,
                tc.tile_pool(name="dns", bufs=2, space="PSUM") as dns,
            ):
                iters = [(h, b) for h in range(HPC) for b in range(NBLK)]
                st = {}
                qtiles = {}

                def load_q(h):
                    qtiles[h] = qp.tile([128, N], BF16, tag="qTh", name="qTh")
                    nc.sync.dma_start(qtiles[h][:, :], qT_d.ap()[h])

                # blob (w01 weights) first — the PE needs it immediately;
                # then qT(0)/k/v; m01 is only needed once exp(0) lands.
                ktile = kvp.tile([128, 32 * 128], BF16)
                vtile = kvp.tile([128, 32 * 128], BF16)
                nc.sync.dma_start(blob[:, :], blob_d.ap())
                nc.sync.dma_start(biaskv[:, :], bias_d.ap())
                load_q(0)
                nc.sync.dma_start(ktile[:, :], k_d.ap())
                nc.sync.dma_start(vtile[:, :], v_d.ap())
                nc.sync.dma_start(m01[:, :], m01_d.ap())
                nc.vector.memset(onesc[:, :], 1.0)

                # ---- compression ----
                # free layout (t, a): pkT[d, 2t+a] = P_a[t].  Prologue psum
                # comes from the sps pool (same shape/tag) so everything
                # fits the 8 psum banks; the transposes reuse pkT/pvT's
                # banks via tag cycling, with a bf16 bitcast view.
                pkT = sps.tile([128, 512], F32, tag="sT", name="pkT")
                pvT = sps.tile([128, 512], F32, tag="sT", name="pvT")
                for c in range(32):
                    nc.tensor.matmul(
                        pkT[:, 16 * c : 16 * c + 16],
                        ktile[:, 128 * c : 128 * (c + 1)],
                        w01k[:, :],
                        start=True, stop=True,
                    )
                # kcT[d,m] = P0[m] + P1[m+1] + bias_k[d]; these DVE adds run
                # while the PE does the v matmuls below.
                pk3 = pkT[:, :].rearrange("p (t a) -> p t a", a=2)
                nc.vector.tensor_scalar_add(kcT[:, 0:M], pk3[:, 0:M, 0], biask[:, 0:1])
                nc.vector.tensor_add(kcT[:, 0:M], kcT[:, 0:M], pk3[:, 1 : M + 1, 1])
                for c in range(32):
                    nc.tensor.matmul(
                        pvT[:, 16 * c : 16 * c + 16],
                        vtile[:, 128 * c : 128 * (c + 1)],
                        w01v[:, :],
                        start=True, stop=True,
                    )
                pv3 = pvT[:, :].rearrange("p (t a) -> p t a", a=2)
                nc.vector.tensor_scalar_add(vcT[:, 0:M], pv3[:, 0:M, 0], biasv[:, 0:1])
                nc.vector.tensor_add(vcT[:, 0:M], vcT[:, 0:M], pv3[:, 1 : M + 1, 1])
                nc.vector.memset(vcT[:, M : M + 1], 0.0)

                def finish_compression():
                    # transpose vcT -> natural vc [m, d] (PV-T stationaries);
                    # emitted after the first two stageA's so QK(0) doesn't
                    # wait behind the transposes' vcT dependency.
                    tpA = sps.tile([128, 512], F32, tag="sT", name="tpA")
                    tpB = sps.tile([128, 512], F32, tag="sT", name="tpB")
                    tpAv = tpA.bitcast(BF16)[:, 0:128]
                    tpBv = tpB.bitcast(BF16)[:, 0:128]
                    nc.tensor.transpose(tpAv, vcT[:, 0:128], ident[:, :])
                    nc.tensor.transpose(tpBv, vcT[:, 128:256], ident[:, :])
                    nc.vector.tensor_copy(vc0[:, :], tpAv)
                    nc.vector.tensor_copy(vc1[:, :], tpBv)

                def stageA(i):
                    h, b = iters[i]
                    if b == 0 and h + 1 < HPC:
                        load_q(h + 1)       # prefetch next head
                    mr = min(32 * b + 31, M)      # visible m count
                    c0r = min(mr, 128)
                    c1r = mr - 128
                    qs = qtiles[h][:, 512 * b : 512 * (b + 1)]

                    sT0 = sps.tile([128, 512], F32, tag="sT")
                    nc.tensor.matmul(
                        sT0[0:c0r, :], kcT[:, 0:c0r], qs,
                        start=True, stop=True,
                    )
                    eT0 = ep.tile([128, 512], BF16, tag="eT")
                    nc.scalar.activation(
                        eT0[0:c0r, :], sT0[0:c0r, :],
                        mybir.ActivationFunctionType.Exp, scale=SM,
                    )
                    eT1 = None
                    if c1r > 0:
                        sT1 = sps.tile([128, 512], F32, tag="sT")
                        nc.tensor.matmul(
                            sT1[0:c1r, :], kcT[:, 128 : 128 + c1r], qs,
                            start=True, stop=True,
                        )
                        eT1 = ep.tile([128, 512], BF16, tag="eT")
                        nc.scalar.activation(
                            eT1[0:c1r, :], sT1[0:c1r, :],
                            mybir.ActivationFunctionType.Exp, scale=SM,
                        )
    # multiplicative staircase mask: only rows [32b, 32b+31) are
                    # partially masked (one aligned 31-row DVE mul with the
                    # matching m01 variant rows); row 32b-1 just zeroes its
                    # first 15 cols (GpSimd memset).  All other computed rows
                    # are fully visible.
                    ps, pe_ = 32 * b, 32 * b + 31
                    if pe_ <= 128:            # piece lives in chunk0
                        mj = m01[:, 512 * b : 512 * (b + 1)]
                        nc.gpsimd.tensor_mul(
                            eT0[ps:pe_, :], eT0[ps:pe_, :], mj[ps:pe_, :]
                        )
                    else:                     # chunk1 (b >= 4)
                        s1, e1 = ps - 128, pe_ - 128
                        mj = m01[:, 512 * (b - 4) : 512 * (b - 3)]
                        nc.gpsimd.tensor_mul(
                            eT1[s1:e1, :], eT1[s1:e1, :], mj[s1:e1, :]
                        )
                    # row 32b-1 masks cols 0:15; it is the last row of the
                    # aligned window [32b-32, 32b), so multiply that window's
                    # first 15 cols by m01r (zero only in rows p%32==31).
                    pr = 32 * b - 1
                    if 0 <= pr < 128:
                        ws = pr - 31
                        nc.gpsimd.tensor_mul(
                            eT0[ws : ws + 32, 0:15], eT0[ws : ws + 32, 0:15],
                            m01r[ws : ws + 32, 0:15],
                        )
                    elif pr >= 128:
                        ws = pr - 159
                        nc.gpsimd.tensor_mul(
                            eT1[ws : ws + 32, 0:15], eT1[ws : ws + 32, 0:15],
                            m01r[ws : ws + 32, 0:15],
                        )
                    st[i] = (c0r, c1r, eT0, eT1)

                def stageB(i):
                    h, b = iters[i]
                    c0r, c1r, eT0, eT1 = st.pop(i)

                    # denominator row dn[1,512] = sum_m eT[m,:]; 2
                    # consecutive blocks share one psum bank at partition
                    # bases 0/64 (matmul psum out base must be 0/32/64) so
                    # one DVE copy serves 2 blocks.
                    j = b % 2
                    if j == 0:
                        st[("dn", h)] = dns.tile(
                            [128, 512], F32, tag="dn", name="dnt"
                        )
                    dnt = st[("dn", h)]
                    drow = dnt[64 * j : 64 * j + 1, :]
                    nc.tensor.matmul(
                        drow, onesc[0:c0r, 0:1], eT0[0:c0r, :],
                        start=True, stop=(c1r <= 0),
                    )
                    if c1r > 0:
                        nc.tensor.matmul(
                            drow, onesc[0:c1r, 0:1], eT1[0:c1r, :],
                            start=False, stop=True,
                        )

                    # transposed PV: oT[vd,512] = vc^T @ eT
                    pvt = pvs.tile([128, 512], F32, tag="pv")
                    nc.tensor.matmul(
                        pvt[:, :], vc0[0:c0r, :], eT0[0:c0r, :],
                        start=True, stop=(c1r <= 0),
                    )
                    if c1r > 0:
                        nc.tensor.matmul(
                            pvt[:, :], vc1[0:c1r, :], eT1[0:c1r, :],
                            start=False, stop=True,
                        )

                    # psum -> sbuf; 4 consecutive blocks share one [128,2048]
                    # sbuf tile and a single 4KB-per-partition DMA.  dn rows
                    # accumulate into a per-head [65, 2048] tile (psum row 0
                    # -> sbuf row 0, row 64 -> row 64), DMA'd twice per head.
                    jo = b % 4
                    if jo == 0:
                        st[("o", h, b // 4)] = op.tile(
                            [128, 2048], BF16, tag="o", name="o_blk"
                        )
                    o_blk = st[("o", h, b // 4)]
                    nc.vector.tensor_copy(
                        o_blk[:, 512 * jo : 512 * (jo + 1)], pvt[:, :]
                    )
                    if jo == 3:
                        nc.sync.dma_start(o_d.ap()[h, b // 4], o_blk[:, :])
                    if b == 0:
                        st[("dnsb", h)] = dnp.tile(
                            [65, 2048], F32, tag="dnsb", name="dnsb"
                        )
                    if j == 1:
                        dnsb = st[("dnsb", h)]
                        q4 = b // 2
                        nc.vector.tensor_copy(
                            dnsb[:, 512 * q4 : 512 * (q4 + 1)], dnt[0:65, :]
                        )
                        if b == NBLK - 1:
                            nc.sync.dma_start(dn_d.ap()[h, 0], dnsb[0:1, :])
                            nc.sync.dma_start(dn_d.ap()[h, 1], dnsb[64:65, :])

                DEPTH = 2
                for i in range(len(iters)):
                    stageA(i)
                    if i == DEPTH - 1:
                        finish_compression()
                    if i >= DEPTH:
                        stageB(i - DEPTH)
                for i in range(len(iters) - DEPTH, len(iters)):
                    stageB(i)
    nc.compile()
    return nc


def make_consts(w_k, pe_k, w_v, pe_v):
    """Host-side constant tensors fed to every core."""
    f = np.float32
    bf = ml_dtypes.bfloat16
    w01k = np.zeros((128, 16), f)
    w01v = np.zeros((128, 16), f)
    for r in range(128):
        j = r // 16
        s = r % 16
        for a in range(2):
            # column layout (j, a): col = 2*j + a, matching psum (t, a)
            w01k[r, 2 * j + a] = w_k[16 * a + s]
            w01v[r, 2 * j + a] = w_v[16 * a + s]
    biask = (w_k[:, None] * pe_k).sum(0).astype(f)[:, None]  # [128,1]
    biasv = (w_v[:, None] * pe_v).sum(0).astype(f)[:, None]
    # variant v: row p = stair(p - 32v + 32); stair(r): n' >= 16r - 481
    m01 = np.ones((8, 128, 512), f)
    for vv in range(8):
        for p in range(128):
            r = p - 32 * vv + 32
            if 0 <= r < 64:
                lo = 16 * r - 481
                if lo >= 512:
                    m01[vv, p, :] = 0.0
                else:
                    m01[vv, p, : max(lo, 0)] = 0.0
    ident = np.eye(128, dtype=f)
    m01r = np.ones((128, 16), f)
    m01r[31::32, :] = 0.0
    blob = np.hstack([w01k, w01v, m01r, ident])          # [128, 176]
    return {
        "blob": np.ascontiguousarray(blob).astype(bf),
        "biaskv": np.ascontiguousarray(np.hstack([biask, biasv])),
        "m01": np.ascontiguousarray(m01.transpose(1, 0, 2).reshape(128, -1)
                                    ).astype(bf),
    }


def make_in_map(q, k, v, consts, core):
    b, hq = core // 4, core % 4
    g = hq // 2
    bf = ml_dtypes.bfloat16
    qT = np.ascontiguousarray(
        q[b, :, 8 * hq : 8 * (hq + 1), :].transpose(1, 2, 0)
    ).astype(bf)  # [8, D, N]
    # device ktile layout: partition r holds chunks c=0..31 of d-rows, i.e.
    # kk[r, 128c + d] = k[128c + r, d]
    kk = np.ascontiguousarray(
        k[b, :, g, :].reshape(32, 128, 128).transpose(1, 0, 2).reshape(128, -1)
    ).astype(bf)
    vv = np.ascontiguousarray(
        v[b, :, g, :].reshape(32, 128, 128).transpose(1, 0, 2).reshape(128, -1)
    ).astype(bf)
    return {"qT": qT, "kk": kk, "vv": vv, **consts}


_CACHE = {}


def _compiled():
    if "nc" not in _CACHE:
        _CACHE["nc"] = build_program()
    return _CACHE["nc"]


def kernel(q, k, v, w_k, pe_k, w_v, pe_v, _trace=False, _trace_kwargs=None):
    q = np.asarray(q, np.float32)
    k = np.asarray(k, np.float32)
    v = np.asarray(v, np.float32)
    consts = make_consts(
        np.asarray(w_k, np.float32), np.asarray(pe_k, np.float32),
        np.asarray(w_v, np.float32), np.asarray(pe_v, np.float32),
    )
    nc = _compiled()
    in_maps = [make_in_map(q, k, v, consts, c) for c in range(8)]
    kw = {}
    if _trace:
        kw = {"trace": True, **(_trace_kwargs or {})}
    res = run_bass_kernel_spmd(nc, in_maps, core_ids=list(range(8)), **kw)
    out = np.empty((B, N, QH, VD), np.float32)
    for c in range(8):
        b, hq = c // 4, c % 4
        oT = np.asarray(res.results[c]["o"], np.float32)    # [8,2,128,2048]
        dnr = np.asarray(res.results[c]["dn"], np.float32)  # [8,2,2048]
        o = oT.transpose(0, 1, 3, 2).reshape(HPC, N, VD)    # [h, n, vd]
        # dn[h, par, 512*(bb//2)+n'] -> [h, n]; n = 512*bb + n', par = bb%2
        d = dnr.reshape(HPC, 2, 4, 512).transpose(0, 2, 1, 3).reshape(HPC, N)
        o /= np.maximum(d, 1e-30)[:, :, None]
        out[b, :, 8 * hq : 8 * (hq + 1), :] = o.transpose(1, 0, 2)
    _CACHE["last_result"] = res
    return out
